# revision 8
# baseline (speedup 1.0000x reference)
"""Chamfer-loss-overlap kernel for 8 Trainium2 NeuronCores.

Math (per batch element, reference semantics):
    P[i,j] = |x_i|^2 + |y_j|^2 - 2 x_i . y_j          (4096 x 4096)
    a = mean(x_mask * min_i P[i,j])    (min over i, per y-point j)
    b = mean(y_mask * min_j P[i,j])    (min over j, per x-point i)
    out = (a - b)^2

Sharding: batch dim B=8 across the 8 cores (data parallel). Each core
computes its own 4096x4096 distance matrix twice (once per min direction,
so both reductions are along the free dim) and returns the two 4096-long
min vectors. Host applies masks / means in float64 and squares the
difference (scalar combine; the all-reduce of two scalars is trivial).

Device kernel strategy:
  - The distance matrix is produced on TensorE as ONE K=13 bf16 matmul per
    128x512 tile: fp32 x/y are split hi/lo into bf16 (x ~ xh + xl), and the
    |x|^2 / |y|^2 terms ride along as extra contraction rows against ones.
    The per-element error of this split is ~1e-6 relative to P values.
  - Row-mins: ScalarE evacuates half of each PSUM strip to SBUF, then
    VectorE's fused tensor_tensor_reduce computes elementwise min of the
    two halves AND the running row-min in a single 1-elem/cycle pass, so
    VectorE only touches each distance value once per two produced.
"""

import numpy as np
from ml_dtypes import bfloat16

import concourse.bacc as bacc
import concourse.bass as bass
import concourse.mybir as mybir
from concourse import tile

B, N, D = 8, 4096, 3
NCORES = 8
NT = N // 128        # 32 output tiles per pass
HALF = 2048          # PSUM strip width (4 banks)
QW = 512             # one PSUM bank of fp32
K = 13               # contraction rows of the augmented matmul

# Set to False to use plain tensor_reduce(min) on PSUM (no ScalarE help)
# (tensor_tensor_reduce faults TRN2 hardware on this path - do not enable)
USE_TTR = False

_CACHE = {}


def _build_nc():
    dt = mybir.dt
    nc = bacc.Bacc("TRN2", target_bir_lowering=False, debug=False,
                   num_devices=NCORES)

    la_d = nc.dram_tensor("la", [K, N], dt.bfloat16, kind="ExternalInput")
    ra_d = nc.dram_tensor("ra", [K, N], dt.bfloat16, kind="ExternalInput")
    lb_d = nc.dram_tensor("lb", [K, N], dt.bfloat16, kind="ExternalInput")
    rb_d = nc.dram_tensor("rb", [K, N], dt.bfloat16, kind="ExternalInput")
    minsA_d = nc.dram_tensor("minsA", [128, NT], dt.float32,
                             kind="ExternalOutput")
    minsB_d = nc.dram_tensor("minsB", [128, NT], dt.float32,
                             kind="ExternalOutput")

    with tile.TileContext(nc) as tc:
        with (
            tc.tile_pool(name="rows", bufs=1) as rows,
            tc.tile_pool(name="psum", bufs=2, space=bass.MemorySpace.PSUM) as psum,
            tc.tile_pool(name="cpy", bufs=3) as cpy,
            tc.tile_pool(name="dum", bufs=4) as dum,
            tc.tile_pool(name="accs", bufs=1) as accs,
        ):
            la = rows.tile([K, N], dt.bfloat16, tag="la")
            ra = rows.tile([K, N], dt.bfloat16, tag="ra")
            lb = rows.tile([K, N], dt.bfloat16, tag="lb")
            rb = rows.tile([K, N], dt.bfloat16, tag="rb")
            nc.sync.dma_start(la[:], la_d[:])
            nc.sync.dma_start(ra[:], ra_d[:])
            nc.sync.dma_start(lb[:], lb_d[:])
            nc.sync.dma_start(rb[:], rb_d[:])

            accA = accs.tile([128, NT, 2], dt.float32, tag="accA")
            accB = accs.tile([128, NT, 2], dt.float32, tag="accB")

            for L, R, acc in ((la, ra, accA), (lb, rb, accB)):
                for it in range(NT):
                    lhsT = L[:, it * 128:(it + 1) * 128]
                    for h in range(2):
                        ps = psum.tile([128, HALF], dt.float32, tag="ps")
                        for q in range(4):
                            j0 = h * HALF + q * QW
                            nc.tensor.matmul(
                                ps[:, q * QW:(q + 1) * QW],
                                lhsT,
                                R[:, j0:j0 + QW],
                                start=True, stop=True,
                            )
                        if USE_TTR:
                            cp = cpy.tile([128, HALF // 2], dt.float32,
                                          tag="cp")
                            nc.scalar.copy(cp[:], ps[:, HALF // 2:])
                            dummy = dum.tile([128, 1], dt.float32, tag="dm")
                            nc.vector.tensor_tensor_reduce(
                                out=dummy.broadcast_to((128, HALF // 2)),
                                in0=ps[:, 0:HALF // 2],
                                in1=cp[:],
                                scale=1.0,
                                scalar=3.0e38,
                                op0=mybir.AluOpType.min,
                                op1=mybir.AluOpType.min,
                                accum_out=acc[:, it, h:h + 1],
                            )
                        else:
                            nc.vector.tensor_reduce(
                                acc[:, it, h:h + 1],
                                ps[:, :],
                                axis=mybir.AxisListType.X,
                                op=mybir.AluOpType.min,
                            )

            finA = accs.tile([128, NT], dt.float32, tag="finA")
            finB = accs.tile([128, NT], dt.float32, tag="finB")
            nc.vector.tensor_reduce(finA[:], accA[:],
                                    axis=mybir.AxisListType.X,
                                    op=mybir.AluOpType.min)
            nc.vector.tensor_reduce(finB[:], accB[:],
                                    axis=mybir.AxisListType.X,
                                    op=mybir.AluOpType.min)
            nc.sync.dma_start(minsA_d[:], finA[:])
            nc.sync.dma_start(minsB_d[:], finB[:])

    nc.compile()
    return nc


def get_nc():
    if "nc" not in _CACHE:
        _CACHE["nc"] = _build_nc()
    return _CACHE["nc"]


def _make_runner(nc):
    """Build a cached jitted SPMD callable for `nc` (one NEFF on all 8
    cores, per-core inputs sharded along axis 0). Mirrors
    bass2jax.run_bass_via_pjrt's multi-core path, but reusable across
    calls so jax tracing/lowering happens once."""
    import jax
    from jax.sharding import Mesh, PartitionSpec
    from jax.experimental.shard_map import shard_map
    from concourse.bass2jax import (
        _bass_exec_p,
        install_neuronx_cc_hook,
        partition_id_tensor,
    )

    install_neuronx_cc_hook()
    partition_name = (nc.partition_id_tensor.name
                      if nc.partition_id_tensor else None)

    in_names = []
    out_names = []
    out_avals = []
    out_shapes = []
    for alloc in nc.m.functions[0].allocations:
        if not isinstance(alloc, mybir.MemoryLocationSet):
            continue
        name = alloc.memorylocations[0].name
        if alloc.kind == "ExternalInput":
            if name != partition_name:
                in_names.append(name)
        elif alloc.kind == "ExternalOutput":
            shape = tuple(alloc.tensor_shape)
            dtype = mybir.dt.np(alloc.dtype)
            out_avals.append(jax.core.ShapedArray(shape, dtype))
            out_names.append(name)
            out_shapes.append((shape, dtype))
    n_params = len(in_names)
    n_outs = len(out_names)
    all_names = list(in_names) + list(out_names)
    if partition_name is not None:
        all_names.append(partition_name)
    donate = tuple(range(n_params, n_params + n_outs))

    def _body(*args):
        operands = list(args)
        if partition_name is not None:
            operands.append(partition_id_tensor())
        outs = _bass_exec_p.bind(
            *operands,
            out_avals=tuple(out_avals),
            in_names=tuple(all_names),
            out_names=tuple(out_names),
            lowering_input_output_aliases=(),
            sim_require_finite=True,
            sim_require_nnan=True,
            nc=nc,
        )
        return tuple(outs)

    devices = jax.devices()[:NCORES]
    mesh = Mesh(np.asarray(devices), ("core",))
    sharded = jax.jit(
        shard_map(_body, mesh=mesh,
                  in_specs=(PartitionSpec("core"),) * (n_params + n_outs),
                  out_specs=(PartitionSpec("core"),) * n_outs,
                  check_rep=False),
        donate_argnums=donate,
        keep_unused=True,
    )

    def run(in_maps):
        concat_in = [
            np.concatenate([np.asarray(m[name]) for m in in_maps], axis=0)
            for name in in_names
        ]
        concat_zeros = [
            np.zeros((NCORES * s[0], *s[1:]), dt) for s, dt in out_shapes
        ]
        out_arrs = sharded(*concat_in, *concat_zeros)
        return [
            {
                name: np.asarray(out_arrs[i]).reshape(
                    NCORES, *out_shapes[i][0])[c]
                for i, name in enumerate(out_names)
            }
            for c in range(NCORES)
        ]

    return run


def get_runner():
    if "run" not in _CACHE:
        _CACHE["run"] = _make_runner(get_nc())
    return _CACHE["run"]


def _f32(v):
    return np.asarray(v, dtype=np.float32)


def _bf(v):
    return np.asarray(v, dtype=np.float32).astype(bfloat16)


def build_rows(xc, yc):
    """Build the four [13, 4096] bf16 row tensors for one batch element.

    Contraction layout (k : L-row      * R-row):
      0-2 : -2*xh_d  * yh_d
      3-5 : -2*xl_d  * yh_d
      6-8 : -2*xh_d  * yl_d
      9   : sqx_h    * 1
      10  : sqx_l    * 1
      11  : 1        * sqy_h
      12  : 1        * sqy_l
    Pass B swaps the roles of x and y with the identical term multiset, so
    P_B = P_A^T up to fp32 accumulation order.
    """
    def side(v):
        vh = _bf(v)
        vl = _bf(_f32(v) - _f32(vh))
        sq = (np.asarray(v, np.float64) ** 2).sum(-1)
        sqh = _bf(sq)
        sql = _bf(sq - np.float64(1.0) * _f32(sqh).astype(np.float64))
        m2h = _bf(-2.0 * _f32(vh))
        m2l = _bf(-2.0 * _f32(vl))
        return vh, vl, sqh, sql, m2h, m2l

    xh, xl, sqxh, sqxl, m2xh, m2xl = side(xc)
    yh, yl, sqyh, sqyl, m2yh, m2yl = side(yc)
    ones = np.ones((N,), dtype=bfloat16)

    def lrows(m2h, m2l, sqh, sql):
        return np.stack([m2h[:, 0], m2h[:, 1], m2h[:, 2],
                         m2l[:, 0], m2l[:, 1], m2l[:, 2],
                         m2h[:, 0], m2h[:, 1], m2h[:, 2],
                         sqh, sql, ones, ones])

    def rrows(vh, vl, sqh, sql):
        return np.stack([vh[:, 0], vh[:, 1], vh[:, 2],
                         vh[:, 0], vh[:, 1], vh[:, 2],
                         vl[:, 0], vl[:, 1], vl[:, 2],
                         ones, ones, sqh, sql])

    return {
        "la": np.ascontiguousarray(lrows(m2xh, m2xl, sqxh, sqxl)),
        "ra": np.ascontiguousarray(rrows(yh, yl, sqyh, sqyl)),
        "lb": np.ascontiguousarray(lrows(m2yh, m2yl, sqyh, sqyl)),
        "rb": np.ascontiguousarray(rrows(xh, xl, sqxh, sqxl)),
    }


def _mins_to_vec(m):
    # m[p, it] is the min for point index it*128 + p
    return np.asarray(m, np.float64).T.reshape(N)


def kernel(x, y, x_mask, y_mask):
    x = np.asarray(x)
    y = np.asarray(y)
    in_maps = [build_rows(x[c], y[c]) for c in range(B)]
    res = get_runner()(in_maps)

    sa = 0.0
    sb = 0.0
    for c in range(B):
        minsA = _mins_to_vec(res[c]["minsA"])  # min over j, per x-point i
        minsB = _mins_to_vec(res[c]["minsB"])  # min over i, per y-point j
        sa += (np.asarray(x_mask[c], np.float64) * minsB).sum()
        sb += (np.asarray(y_mask[c], np.float64) * minsA).sum()
    a = sa / (B * N)
    b = sb / (B * N)
    return np.asarray((a - b) ** 2, dtype=np.float32)


# revision 10
# speedup vs baseline: 9.1296x; 9.1296x over previous
"""Chamfer-loss-overlap kernel for 8 Trainium2 NeuronCores.

Math (per batch element, reference semantics):
    P[i,j] = |x_i|^2 + |y_j|^2 - 2 x_i . y_j          (4096 x 4096)
    a = mean(x_mask * min_i P[i,j])    (min over i, per y-point j)
    b = mean(y_mask * min_j P[i,j])    (min over j, per x-point i)
    out = (a - b)^2

Sharding: batch dim B=8 across the 8 cores (data parallel). Each core
computes its own 4096x4096 distance matrix twice (once per min direction,
so both reductions are along the free dim) and returns the two 4096-long
min vectors. Host applies masks / means in float64 and squares the
difference (scalar combine; the all-reduce of two scalars is trivial).

Device kernel strategy:
  - The distance matrix is produced on TensorE as ONE K=13 bf16 matmul per
    128x512 tile: fp32 x/y are split hi/lo into bf16 (x ~ xh + xl), and the
    |x|^2 / |y|^2 terms ride along as extra contraction rows against ones.
    The per-element error of this split is ~1e-6 relative to P values.
  - Row-mins: ScalarE evacuates half of each PSUM strip to SBUF, then
    VectorE's fused tensor_tensor_reduce computes elementwise min of the
    two halves AND the running row-min in a single 1-elem/cycle pass, so
    VectorE only touches each distance value once per two produced.
"""

import numpy as np
from ml_dtypes import bfloat16

import concourse.bacc as bacc
import concourse.bass as bass
import concourse.mybir as mybir
from concourse import tile

B, N, D = 8, 4096, 3
NCORES = 8
NT = N // 128        # 32 output tiles per pass
HALF = 2048          # PSUM strip width (4 banks)
QW = 512             # one PSUM bank of fp32
K = 13               # contraction rows of the augmented matmul

# Set to False to use plain tensor_reduce(min) on PSUM (no ScalarE help)
# (tensor_tensor_reduce faults TRN2 hardware on this path - do not enable)
USE_TTR = False

_CACHE = {}


def _build_nc():
    dt = mybir.dt
    nc = bacc.Bacc("TRN2", target_bir_lowering=False, debug=False,
                   num_devices=NCORES)

    la_d = nc.dram_tensor("la", [K, N], dt.bfloat16, kind="ExternalInput")
    ra_d = nc.dram_tensor("ra", [K, N], dt.bfloat16, kind="ExternalInput")
    lb_d = nc.dram_tensor("lb", [K, N], dt.bfloat16, kind="ExternalInput")
    rb_d = nc.dram_tensor("rb", [K, N], dt.bfloat16, kind="ExternalInput")
    minsA_d = nc.dram_tensor("minsA", [128, NT], dt.float32,
                             kind="ExternalOutput")
    minsB_d = nc.dram_tensor("minsB", [128, NT], dt.float32,
                             kind="ExternalOutput")

    with tile.TileContext(nc) as tc:
        with (
            tc.tile_pool(name="rows", bufs=1) as rows,
            tc.tile_pool(name="psum", bufs=2, space=bass.MemorySpace.PSUM) as psum,
            tc.tile_pool(name="cpy", bufs=3) as cpy,
            tc.tile_pool(name="dum", bufs=4) as dum,
            tc.tile_pool(name="accs", bufs=1) as accs,
        ):
            la = rows.tile([K, N], dt.bfloat16, tag="la")
            ra = rows.tile([K, N], dt.bfloat16, tag="ra")
            lb = rows.tile([K, N], dt.bfloat16, tag="lb")
            rb = rows.tile([K, N], dt.bfloat16, tag="rb")
            nc.sync.dma_start(la[:], la_d[:])
            nc.sync.dma_start(ra[:], ra_d[:])
            nc.sync.dma_start(lb[:], lb_d[:])
            nc.sync.dma_start(rb[:], rb_d[:])

            accA = accs.tile([128, NT, 2], dt.float32, tag="accA")
            accB = accs.tile([128, NT, 2], dt.float32, tag="accB")

            for L, R, acc in ((la, ra, accA), (lb, rb, accB)):
                for it in range(NT):
                    lhsT = L[:, it * 128:(it + 1) * 128]
                    for h in range(2):
                        ps = psum.tile([128, HALF], dt.float32, tag="ps")
                        for q in range(4):
                            j0 = h * HALF + q * QW
                            nc.tensor.matmul(
                                ps[:, q * QW:(q + 1) * QW],
                                lhsT,
                                R[:, j0:j0 + QW],
                                start=True, stop=True,
                            )
                        if USE_TTR:
                            cp = cpy.tile([128, HALF // 2], dt.float32,
                                          tag="cp")
                            nc.scalar.copy(cp[:], ps[:, HALF // 2:])
                            dummy = dum.tile([128, 1], dt.float32, tag="dm")
                            nc.vector.tensor_tensor_reduce(
                                out=dummy.broadcast_to((128, HALF // 2)),
                                in0=ps[:, 0:HALF // 2],
                                in1=cp[:],
                                scale=1.0,
                                scalar=3.0e38,
                                op0=mybir.AluOpType.min,
                                op1=mybir.AluOpType.min,
                                accum_out=acc[:, it, h:h + 1],
                            )
                        else:
                            nc.vector.tensor_reduce(
                                acc[:, it, h:h + 1],
                                ps[:, :],
                                axis=mybir.AxisListType.X,
                                op=mybir.AluOpType.min,
                            )

            finA = accs.tile([128, NT], dt.float32, tag="finA")
            finB = accs.tile([128, NT], dt.float32, tag="finB")
            nc.vector.tensor_reduce(finA[:], accA[:],
                                    axis=mybir.AxisListType.X,
                                    op=mybir.AluOpType.min)
            nc.vector.tensor_reduce(finB[:], accB[:],
                                    axis=mybir.AxisListType.X,
                                    op=mybir.AluOpType.min)
            nc.sync.dma_start(minsA_d[:], finA[:])
            nc.sync.dma_start(minsB_d[:], finB[:])

    nc.compile()
    return nc


def get_nc():
    if "nc" not in _CACHE:
        _CACHE["nc"] = _build_nc()
    return _CACHE["nc"]


def _make_runner(nc):
    """Build a cached jitted SPMD callable for `nc` (one NEFF on all 8
    cores, per-core inputs sharded along axis 0). Mirrors
    bass2jax.run_bass_via_pjrt's multi-core path, but reusable across
    calls so jax tracing/lowering happens once."""
    import jax
    from jax.sharding import Mesh, PartitionSpec
    from jax.experimental.shard_map import shard_map
    from concourse.bass2jax import (
        _bass_exec_p,
        install_neuronx_cc_hook,
        partition_id_tensor,
    )

    install_neuronx_cc_hook()
    partition_name = (nc.partition_id_tensor.name
                      if nc.partition_id_tensor else None)

    in_names = []
    out_names = []
    out_avals = []
    out_shapes = []
    for alloc in nc.m.functions[0].allocations:
        if not isinstance(alloc, mybir.MemoryLocationSet):
            continue
        name = alloc.memorylocations[0].name
        if alloc.kind == "ExternalInput":
            if name != partition_name:
                in_names.append(name)
        elif alloc.kind == "ExternalOutput":
            shape = tuple(alloc.tensor_shape)
            dtype = mybir.dt.np(alloc.dtype)
            out_avals.append(jax.core.ShapedArray(shape, dtype))
            out_names.append(name)
            out_shapes.append((shape, dtype))
    n_params = len(in_names)
    n_outs = len(out_names)
    all_names = list(in_names) + list(out_names)
    if partition_name is not None:
        all_names.append(partition_name)
    donate = tuple(range(n_params, n_params + n_outs))

    def _body(*args):
        operands = list(args)
        if partition_name is not None:
            operands.append(partition_id_tensor())
        outs = _bass_exec_p.bind(
            *operands,
            out_avals=tuple(out_avals),
            in_names=tuple(all_names),
            out_names=tuple(out_names),
            lowering_input_output_aliases=(),
            sim_require_finite=True,
            sim_require_nnan=True,
            nc=nc,
        )
        return tuple(outs)

    devices = jax.devices()[:NCORES]
    mesh = Mesh(np.asarray(devices), ("core",))
    sharded = jax.jit(
        shard_map(_body, mesh=mesh,
                  in_specs=(PartitionSpec("core"),) * (n_params + n_outs),
                  out_specs=(PartitionSpec("core"),) * n_outs,
                  check_rep=False),
        donate_argnums=donate,
        keep_unused=True,
    )

    def prep(in_maps):
        concat_in = [
            np.concatenate([np.asarray(m[name]) for m in in_maps], axis=0)
            for name in in_names
        ]
        return concat_in

    def exec_prepped(concat_in):
        concat_zeros = [
            np.zeros((NCORES * s[0], *s[1:]), dt) for s, dt in out_shapes
        ]
        return sharded(*concat_in, *concat_zeros)

    def unpack(out_arrs):
        return [
            {
                name: np.asarray(out_arrs[i]).reshape(
                    NCORES, *out_shapes[i][0])[c]
                for i, name in enumerate(out_names)
            }
            for c in range(NCORES)
        ]

    def run(in_maps):
        return unpack(exec_prepped(prep(in_maps)))

    run.prep = prep
    run.exec_prepped = exec_prepped
    run.unpack = unpack
    run.mesh = mesh
    return run


def get_runner():
    if "run" not in _CACHE:
        _CACHE["run"] = _make_runner(get_nc())
    return _CACHE["run"]


def _f32(v):
    return np.asarray(v, dtype=np.float32)


def _bf(v):
    return np.asarray(v, dtype=np.float32).astype(bfloat16)


def build_rows(xc, yc):
    """Build the four [13, 4096] bf16 row tensors for one batch element.

    Contraction layout (k : L-row      * R-row):
      0-2 : -2*xh_d  * yh_d
      3-5 : -2*xl_d  * yh_d
      6-8 : -2*xh_d  * yl_d
      9   : sqx_h    * 1
      10  : sqx_l    * 1
      11  : 1        * sqy_h
      12  : 1        * sqy_l
    Pass B swaps the roles of x and y with the identical term multiset, so
    P_B = P_A^T up to fp32 accumulation order.
    """
    def side(v):
        vh = _bf(v)
        vl = _bf(_f32(v) - _f32(vh))
        sq = (np.asarray(v, np.float64) ** 2).sum(-1)
        sqh = _bf(sq)
        sql = _bf(sq - np.float64(1.0) * _f32(sqh).astype(np.float64))
        m2h = _bf(-2.0 * _f32(vh))
        m2l = _bf(-2.0 * _f32(vl))
        return vh, vl, sqh, sql, m2h, m2l

    xh, xl, sqxh, sqxl, m2xh, m2xl = side(xc)
    yh, yl, sqyh, sqyl, m2yh, m2yl = side(yc)
    ones = np.ones((N,), dtype=bfloat16)

    def lrows(m2h, m2l, sqh, sql):
        return np.stack([m2h[:, 0], m2h[:, 1], m2h[:, 2],
                         m2l[:, 0], m2l[:, 1], m2l[:, 2],
                         m2h[:, 0], m2h[:, 1], m2h[:, 2],
                         sqh, sql, ones, ones])

    def rrows(vh, vl, sqh, sql):
        return np.stack([vh[:, 0], vh[:, 1], vh[:, 2],
                         vh[:, 0], vh[:, 1], vh[:, 2],
                         vl[:, 0], vl[:, 1], vl[:, 2],
                         ones, ones, sqh, sql])

    return {
        "la": np.ascontiguousarray(lrows(m2xh, m2xl, sqxh, sqxl)),
        "ra": np.ascontiguousarray(rrows(yh, yl, sqyh, sqyl)),
        "lb": np.ascontiguousarray(lrows(m2yh, m2yl, sqyh, sqyl)),
        "rb": np.ascontiguousarray(rrows(xh, xl, sqxh, sqxl)),
    }


def _mins_to_vec(m):
    # m[p, it] is the min for point index it*128 + p
    return np.asarray(m, np.float64).T.reshape(N)


def kernel(x, y, x_mask, y_mask):
    x = np.asarray(x)
    y = np.asarray(y)
    in_maps = [build_rows(x[c], y[c]) for c in range(B)]
    res = get_runner()(in_maps)

    sa = 0.0
    sb = 0.0
    for c in range(B):
        minsA = _mins_to_vec(res[c]["minsA"])  # min over j, per x-point i
        minsB = _mins_to_vec(res[c]["minsB"])  # min over i, per y-point j
        sa += (np.asarray(x_mask[c], np.float64) * minsB).sum()
        sb += (np.asarray(y_mask[c], np.float64) * minsA).sum()
    a = sa / (B * N)
    b = sb / (B * N)
    return np.asarray((a - b) ** 2, dtype=np.float32)


# revision 27
# speedup vs baseline: 316.0491x; 34.6182x over previous
"""Chamfer-loss-overlap kernel for 8 Trainium2 NeuronCores.

Math (per batch element, reference semantics):
    P[i,j] = |x_i|^2 + |y_j|^2 - 2 x_i . y_j          (4096 x 4096)
    a = mean(x_mask * min_i P[i,j])    (min over i, per y-point j)
    b = mean(y_mask * min_j P[i,j])    (min over j, per x-point i)
    out = (a - b)^2

Sharding: batch dim B=8 across the 8 cores (data parallel). Each core
computes its own 4096x4096 distance matrix twice (once per min direction,
so both reductions are along the free dim) and returns the two 4096-long
min vectors. Host applies masks / means in float64 and squares the
difference (scalar combine; the all-reduce of two scalars is trivial).

Device kernel strategy:
  - The distance matrix is produced on TensorE as ONE K=13 bf16 matmul per
    128x512 tile: fp32 x/y are split hi/lo into bf16 (x ~ xh + xl), and the
    |x|^2 / |y|^2 terms ride along as extra contraction rows against ones.
    The per-element error of this split is ~1e-6 relative to P values.
  - Row-mins: ScalarE casts each PSUM strip to fp16 in SBUF; VectorE
    reduce-min consumes the 16-bit copy. Final fold + DMA of the two
    [128, 32] min tensors; masked means run on the host in float64.
"""

import numpy as np
from ml_dtypes import bfloat16

import concourse.bacc as bacc
import concourse.bass as bass
import concourse.mybir as mybir
from concourse import tile

B, N, D = 8, 4096, 3
NCORES = 8
NT = N // 128        # 32 output tiles per pass
QW = 512             # one PSUM bank of fp32
K = 13               # contraction rows of the augmented matmul

# PSUM strip geometry: SW-wide strips, PSUM_BUFS in flight
# (SW * PSUM_BUFS * 4B must be <= 16KB per partition = 8 banks)
SW = 2048
PSUM_BUFS = 2
NSTRIP = N // SW
CPY_BUFS = 3

# Reduction strategy:
#   "cast16b":  ScalarE casts each PSUM strip to bf16 in SBUF (fast 2x+
#               cast), VectorE reduce-min runs at 2x on bf16 (fastest,
#               ~4e-3 extra rel err from bf16 min values)
#   "cast16":   same but fp16 (better precision; ACT cast and reduce both
#               drop to 1x -> slower)
#   "red_psum": VectorE reduce-min directly from PSUM at 1x (safe fallback)
# (tensor_tensor_reduce faults TRN2 hardware on this path - do not use)
REDUCE_MODE = "cast16"

# Pack the 4 matmuls of each PSUM strip into the 4 32-row groups of the PE
# array (K=13 fits in one group), so they run concurrently (~3x TensorE).
ROW_PACK = False

_CACHE = {}


def _build_nc(reps=1):
    dt = mybir.dt
    nc = bacc.Bacc("TRN2", target_bir_lowering=False, debug=False,
                   num_devices=NCORES)

    la_d = nc.dram_tensor("la", [K, N], dt.bfloat16, kind="ExternalInput")
    ra_d = nc.dram_tensor("ra", [K, N], dt.bfloat16, kind="ExternalInput")
    lb_d = nc.dram_tensor("lb", [K, N], dt.bfloat16, kind="ExternalInput")
    rb_d = nc.dram_tensor("rb", [K, N], dt.bfloat16, kind="ExternalInput")
    minsA_d = nc.dram_tensor("minsA", [128, NT], dt.float32,
                             kind="ExternalOutput")
    minsB_d = nc.dram_tensor("minsB", [128, NT], dt.float32,
                             kind="ExternalOutput")

    with tile.TileContext(nc) as tc:
        with (
            tc.tile_pool(name="rows", bufs=1) as rows,
            tc.tile_pool(name="accs", bufs=1) as accs,
        ):
            npart = 96 + K if ROW_PACK else K
            la = rows.tile([npart, N], dt.bfloat16, tag="la")
            ra = rows.tile([npart, N], dt.bfloat16, tag="ra")
            lb = rows.tile([npart, N], dt.bfloat16, tag="lb")
            rb = rows.tile([npart, N], dt.bfloat16, tag="rb")
            for t, d in ((la, la_d), (ra, ra_d), (lb, lb_d), (rb, rb_d)):
                if ROW_PACK:
                    for r in range(4):
                        nc.sync.dma_start(t[32 * r:32 * r + K, :], d[:])
                else:
                    nc.sync.dma_start(t[:], d[:])

            accA = accs.tile([128, NT, NSTRIP], dt.float32, tag="accA")
            accB = accs.tile([128, NT, NSTRIP], dt.float32, tag="accB")

            import contextlib
            rep_ctx = (tc.For_i(0, reps, 1) if reps > 1
                       else contextlib.nullcontext())
            with rep_ctx:
                _emit_main(nc, tc, la, ra, lb, rb, accA, accB)

            finA = accs.tile([128, NT], dt.float32, tag="finA")
            finB = accs.tile([128, NT], dt.float32, tag="finB")
            nc.vector.tensor_reduce(finA[:], accA[:],
                                    axis=mybir.AxisListType.X,
                                    op=mybir.AluOpType.min)
            nc.vector.tensor_reduce(finB[:], accB[:],
                                    axis=mybir.AxisListType.X,
                                    op=mybir.AluOpType.min)
            nc.sync.dma_start(minsA_d[:], finA[:])
            nc.sync.dma_start(minsB_d[:], finB[:])

    nc.compile()
    return nc


def _emit_main(nc, tc, la, ra, lb, rb, accA, accB):
    dt = mybir.dt
    with (
        tc.tile_pool(name="psum", bufs=PSUM_BUFS,
                     space=bass.MemorySpace.PSUM) as psum,
        tc.tile_pool(name="cpy", bufs=CPY_BUFS) as cpy,
    ):
        for L, R, acc in ((la, ra, accA), (lb, rb, accB)):
            for it in range(NT):
                i0 = it * 128
                for h in range(NSTRIP):
                    ps = psum.tile([128, SW], dt.float32, tag="ps", name="ps")
                    for q in range(SW // QW):
                        j0 = h * SW + q * QW
                        nc.tensor.matmul(
                            ps[:, q * QW:(q + 1) * QW],
                            L[:, i0:i0 + 128],
                            R[:, j0:j0 + QW],
                            start=True, stop=True,
                        )
                    if REDUCE_MODE in ("cast16", "cast16b"):
                        cdt = (dt.bfloat16 if REDUCE_MODE == "cast16b"
                               else dt.float16)
                        cp = cpy.tile([128, SW], cdt, tag="cp", name="cp")
                        nc.scalar.copy(cp[:], ps[:, :])
                        nc.vector.tensor_reduce(
                            acc[:, it, h:h + 1],
                            cp[:],
                            axis=mybir.AxisListType.X,
                            op=mybir.AluOpType.min,
                        )
                    else:
                        nc.vector.tensor_reduce(
                            acc[:, it, h:h + 1],
                            ps[:, :],
                            axis=mybir.AxisListType.X,
                            op=mybir.AluOpType.min,
                        )


def get_nc():
    if "nc" not in _CACHE:
        _CACHE["nc"] = _build_nc()
    return _CACHE["nc"]


def _make_runner(nc):
    """Build a cached jitted SPMD callable for `nc` (one NEFF on all 8
    cores, per-core inputs sharded along axis 0). Mirrors
    bass2jax.run_bass_via_pjrt's multi-core path, but reusable across
    calls so jax tracing/lowering happens once."""
    import jax
    from jax.sharding import Mesh, PartitionSpec
    from jax.experimental.shard_map import shard_map
    from concourse.bass2jax import (
        _bass_exec_p,
        install_neuronx_cc_hook,
        partition_id_tensor,
    )

    install_neuronx_cc_hook()
    partition_name = (nc.partition_id_tensor.name
                      if nc.partition_id_tensor else None)

    in_names = []
    out_names = []
    out_avals = []
    out_shapes = []
    for alloc in nc.m.functions[0].allocations:
        if not isinstance(alloc, mybir.MemoryLocationSet):
            continue
        name = alloc.memorylocations[0].name
        if alloc.kind == "ExternalInput":
            if name != partition_name:
                in_names.append(name)
        elif alloc.kind == "ExternalOutput":
            shape = tuple(alloc.tensor_shape)
            dtype = mybir.dt.np(alloc.dtype)
            out_avals.append(jax.core.ShapedArray(shape, dtype))
            out_names.append(name)
            out_shapes.append((shape, dtype))
    n_params = len(in_names)
    n_outs = len(out_names)
    all_names = list(in_names) + list(out_names)
    if partition_name is not None:
        all_names.append(partition_name)
    donate = tuple(range(n_params, n_params + n_outs))

    def _body(*args):
        operands = list(args)
        if partition_name is not None:
            operands.append(partition_id_tensor())
        outs = _bass_exec_p.bind(
            *operands,
            out_avals=tuple(out_avals),
            in_names=tuple(all_names),
            out_names=tuple(out_names),
            lowering_input_output_aliases=(),
            sim_require_finite=True,
            sim_require_nnan=True,
            nc=nc,
        )
        return tuple(outs)

    devices = jax.devices()[:NCORES]
    mesh = Mesh(np.asarray(devices), ("core",))
    sharded = jax.jit(
        shard_map(_body, mesh=mesh,
                  in_specs=(PartitionSpec("core"),) * (n_params + n_outs),
                  out_specs=(PartitionSpec("core"),) * n_outs,
                  check_rep=False),
        donate_argnums=donate,
        keep_unused=True,
    )

    def prep(in_maps):
        concat_in = [
            np.concatenate([np.asarray(m[name]) for m in in_maps], axis=0)
            for name in in_names
        ]
        return concat_in

    def exec_prepped(concat_in):
        concat_zeros = [
            np.zeros((NCORES * s[0], *s[1:]), dt) for s, dt in out_shapes
        ]
        return sharded(*concat_in, *concat_zeros)

    def unpack(out_arrs):
        return [
            {
                name: np.asarray(out_arrs[i]).reshape(
                    NCORES, *out_shapes[i][0])[c]
                for i, name in enumerate(out_names)
            }
            for c in range(NCORES)
        ]

    def run(in_maps):
        return unpack(exec_prepped(prep(in_maps)))

    run.prep = prep
    run.exec_prepped = exec_prepped
    run.unpack = unpack
    run.mesh = mesh
    return run


def get_runner():
    if "run" not in _CACHE:
        _CACHE["run"] = _make_runner(get_nc())
    return _CACHE["run"]


def _f32(v):
    return np.asarray(v, dtype=np.float32)


def _bf(v):
    return np.asarray(v, dtype=np.float32).astype(bfloat16)


def build_rows(xc, yc):
    """Build the four [13, 4096] bf16 row tensors for one batch element.

    Contraction layout (k : L-row      * R-row):
      0-2 : -2*xh_d  * yh_d
      3-5 : -2*xl_d  * yh_d
      6-8 : -2*xh_d  * yl_d
      9   : sqx_h    * 1
      10  : sqx_l    * 1
      11  : 1        * sqy_h
      12  : 1        * sqy_l
    Pass B swaps the roles of x and y with the identical term multiset, so
    P_B = P_A^T up to fp32 accumulation order.
    """
    def side(v):
        vh = _bf(v)
        vl = _bf(_f32(v) - _f32(vh))
        sq = (np.asarray(v, np.float64) ** 2).sum(-1)
        sqh = _bf(sq)
        sql = _bf(sq - np.float64(1.0) * _f32(sqh).astype(np.float64))
        m2h = _bf(-2.0 * _f32(vh))
        m2l = _bf(-2.0 * _f32(vl))
        return vh, vl, sqh, sql, m2h, m2l

    xh, xl, sqxh, sqxl, m2xh, m2xl = side(xc)
    yh, yl, sqyh, sqyl, m2yh, m2yl = side(yc)
    ones = np.ones((N,), dtype=bfloat16)

    def lrows(m2h, m2l, sqh, sql):
        return np.stack([m2h[:, 0], m2h[:, 1], m2h[:, 2],
                         m2l[:, 0], m2l[:, 1], m2l[:, 2],
                         m2h[:, 0], m2h[:, 1], m2h[:, 2],
                         sqh, sql, ones, ones])

    def rrows(vh, vl, sqh, sql):
        return np.stack([vh[:, 0], vh[:, 1], vh[:, 2],
                         vh[:, 0], vh[:, 1], vh[:, 2],
                         vl[:, 0], vl[:, 1], vl[:, 2],
                         ones, ones, sqh, sql])

    return {
        "la": np.ascontiguousarray(lrows(m2xh, m2xl, sqxh, sqxl)),
        "ra": np.ascontiguousarray(rrows(yh, yl, sqyh, sqyl)),
        "lb": np.ascontiguousarray(lrows(m2yh, m2yl, sqyh, sqyl)),
        "rb": np.ascontiguousarray(rrows(xh, xl, sqxh, sqxl)),
    }


def _mins_to_vec(m):
    # m[p, it] is the min for point index it*128 + p
    return np.asarray(m, np.float64).T.reshape(N)


def kernel(x, y, x_mask, y_mask):
    x = np.asarray(x)
    y = np.asarray(y)
    in_maps = [build_rows(x[c], y[c]) for c in range(B)]
    res = get_runner()(in_maps)

    sa = 0.0
    sb = 0.0
    for c in range(B):
        minsA = _mins_to_vec(res[c]["minsA"])  # min over j, per x-point i
        minsB = _mins_to_vec(res[c]["minsB"])  # min over i, per y-point j
        sa += (np.asarray(x_mask[c], np.float64) * minsB).sum()
        sb += (np.asarray(y_mask[c], np.float64) * minsA).sum()
    a = sa / (B * N)
    b = sb / (B * N)
    return np.asarray((a - b) ** 2, dtype=np.float32)


# revision 32
# speedup vs baseline: 318.2352x; 1.0069x over previous
"""Chamfer-loss-overlap kernel for 8 Trainium2 NeuronCores.

Math (per batch element, reference semantics):
    P[i,j] = |x_i|^2 + |y_j|^2 - 2 x_i . y_j          (4096 x 4096)
    a = mean(x_mask * min_i P[i,j])    (min over i, per y-point j)
    b = mean(y_mask * min_j P[i,j])    (min over j, per x-point i)
    out = (a - b)^2

Sharding: batch dim B=8 across the 8 cores (data parallel). Each core
computes its own 4096x4096 distance matrix twice (once per min direction,
so both reductions are along the free dim) and returns the two 4096-long
min vectors. Host applies masks / means in float64 and squares the
difference (scalar combine; the all-reduce of two scalars is trivial).

Device kernel strategy:
  - The distance matrix is produced on TensorE as ONE K=13 bf16 matmul per
    128x512 tile: fp32 x/y are split hi/lo into bf16 (x ~ xh + xl), and the
    |x|^2 / |y|^2 terms ride along as extra contraction rows against ones.
    The per-element error of this split is ~1e-6 relative to P values.
  - Row-mins: ScalarE casts each PSUM strip to fp16 in SBUF; VectorE
    reduce-min consumes the 16-bit copy. Final fold + DMA of the two
    [128, 32] min tensors; masked means run on the host in float64.
"""

import numpy as np
from ml_dtypes import bfloat16

import concourse.bacc as bacc
import concourse.bass as bass
import concourse.mybir as mybir
from concourse import tile

B, N, D = 8, 4096, 3
NCORES = 8
NT = N // 128        # 32 output tiles per pass
QW = 512             # one PSUM bank of fp32
K = 13               # contraction rows of the augmented matmul

# PSUM strip geometry: SW-wide strips, PSUM_BUFS in flight
# (SW * PSUM_BUFS * 4B must be <= 16KB per partition = 8 banks)
SW = 2048
PSUM_BUFS = 2
NSTRIP = N // SW
CPY_BUFS = 3

# Reduction strategy:
#   "cast16b":  ScalarE casts each PSUM strip to bf16 in SBUF (fast 2x+
#               cast), VectorE reduce-min runs at 2x on bf16 (fastest,
#               ~4e-3 extra rel err from bf16 min values)
#   "cast16":   same but fp16 (better precision; ACT cast and reduce both
#               drop to 1x -> slower)
#   "red_psum": VectorE reduce-min directly from PSUM at 1x (safe fallback)
# (tensor_tensor_reduce faults TRN2 hardware on this path - do not use)
REDUCE_MODE = "cast16"

# Pack the 4 matmuls of each PSUM strip into the 4 32-row groups of the PE
# array (K=13 fits in one group), so they run concurrently (~3x TensorE).
ROW_PACK = False

_CACHE = {}


def _build_nc(reps=1):
    dt = mybir.dt
    nc = bacc.Bacc("TRN2", target_bir_lowering=False, debug=False,
                   num_devices=NCORES)

    la_d = nc.dram_tensor("la", [K, N], dt.bfloat16, kind="ExternalInput")
    ra_d = nc.dram_tensor("ra", [K, N], dt.bfloat16, kind="ExternalInput")
    lb_d = nc.dram_tensor("lb", [K, N], dt.bfloat16, kind="ExternalInput")
    rb_d = nc.dram_tensor("rb", [K, N], dt.bfloat16, kind="ExternalInput")
    minsA_d = nc.dram_tensor("minsA", [128, NT], dt.float32,
                             kind="ExternalOutput")
    minsB_d = nc.dram_tensor("minsB", [128, NT], dt.float32,
                             kind="ExternalOutput")

    with tile.TileContext(nc) as tc:
        with (
            tc.tile_pool(name="rows", bufs=1) as rows,
            tc.tile_pool(name="accs", bufs=1) as accs,
        ):
            npart = 96 + K if ROW_PACK else K
            la = rows.tile([npart, N], dt.bfloat16, tag="la")
            ra = rows.tile([npart, N], dt.bfloat16, tag="ra")
            lb = rows.tile([npart, N], dt.bfloat16, tag="lb")
            rb = rows.tile([npart, N], dt.bfloat16, tag="rb")
            for t, d in ((la, la_d), (ra, ra_d), (lb, lb_d), (rb, rb_d)):
                if ROW_PACK:
                    for r in range(4):
                        nc.sync.dma_start(t[32 * r:32 * r + K, :], d[:])
                else:
                    nc.sync.dma_start(t[:], d[:])

            acc_d = NSTRIP
            accA = accs.tile([128, NT, acc_d], dt.float32, tag="accA")
            accB = accs.tile([128, NT, acc_d], dt.float32, tag="accB")

            import contextlib
            rep_ctx = (tc.For_i(0, reps, 1) if reps > 1
                       else contextlib.nullcontext())
            with rep_ctx:
                _emit_main(nc, tc, la, ra, lb, rb, accA, accB)

            finA = accs.tile([128, NT], dt.float32, tag="finA")
            finB = accs.tile([128, NT], dt.float32, tag="finB")
            nc.vector.tensor_reduce(finA[:], accA[:],
                                    axis=mybir.AxisListType.X,
                                    op=mybir.AluOpType.min)
            nc.vector.tensor_reduce(finB[:], accB[:],
                                    axis=mybir.AxisListType.X,
                                    op=mybir.AluOpType.min)
            nc.sync.dma_start(minsA_d[:], finA[:])
            nc.sync.dma_start(minsB_d[:], finB[:])

    nc.compile()
    return nc


def _emit_main(nc, tc, la, ra, lb, rb, accA, accB):
    dt = mybir.dt
    with (
        tc.tile_pool(name="psum", bufs=PSUM_BUFS,
                     space=bass.MemorySpace.PSUM) as psum,
        tc.tile_pool(name="cpy", bufs=CPY_BUFS) as cpy,
    ):
        cdt = dt.bfloat16 if REDUCE_MODE == "cast16b" else dt.float16
        for L, R, acc in ((la, ra, accA), (lb, rb, accB)):
            for it in range(NT):
                i0 = it * 128
                for h in range(NSTRIP):
                    ps = psum.tile([128, SW], dt.float32, tag="ps", name="ps")
                    for q in range(SW // QW):
                        j0 = h * SW + q * QW
                        nc.tensor.matmul(
                            ps[:, q * QW:(q + 1) * QW],
                            L[:, i0:i0 + 128],
                            R[:, j0:j0 + QW],
                            start=True, stop=True,
                        )
                    if REDUCE_MODE in ("cast16", "cast16b"):
                        cp = cpy.tile([128, SW], cdt, tag="cp", name="cp")
                        nc.scalar.copy(cp[:], ps[:, :])
                        nc.vector.tensor_reduce(
                            acc[:, it, h:h + 1],
                            cp[:],
                            axis=mybir.AxisListType.X,
                            op=mybir.AluOpType.min,
                        )
                    else:
                        nc.vector.tensor_reduce(
                            acc[:, it, h:h + 1],
                            ps[:, :],
                            axis=mybir.AxisListType.X,
                            op=mybir.AluOpType.min,
                        )


def get_nc():
    if "nc" not in _CACHE:
        _CACHE["nc"] = _build_nc()
    return _CACHE["nc"]


def _make_runner(nc):
    """Build a cached jitted SPMD callable for `nc` (one NEFF on all 8
    cores, per-core inputs sharded along axis 0). Mirrors
    bass2jax.run_bass_via_pjrt's multi-core path, but reusable across
    calls so jax tracing/lowering happens once."""
    import jax
    from jax.sharding import Mesh, PartitionSpec
    from jax.experimental.shard_map import shard_map
    from concourse.bass2jax import (
        _bass_exec_p,
        install_neuronx_cc_hook,
        partition_id_tensor,
    )

    install_neuronx_cc_hook()
    partition_name = (nc.partition_id_tensor.name
                      if nc.partition_id_tensor else None)

    in_names = []
    out_names = []
    out_avals = []
    out_shapes = []
    for alloc in nc.m.functions[0].allocations:
        if not isinstance(alloc, mybir.MemoryLocationSet):
            continue
        name = alloc.memorylocations[0].name
        if alloc.kind == "ExternalInput":
            if name != partition_name:
                in_names.append(name)
        elif alloc.kind == "ExternalOutput":
            shape = tuple(alloc.tensor_shape)
            dtype = mybir.dt.np(alloc.dtype)
            out_avals.append(jax.core.ShapedArray(shape, dtype))
            out_names.append(name)
            out_shapes.append((shape, dtype))
    n_params = len(in_names)
    n_outs = len(out_names)
    all_names = list(in_names) + list(out_names)
    if partition_name is not None:
        all_names.append(partition_name)
    donate = tuple(range(n_params, n_params + n_outs))

    def _body(*args):
        operands = list(args)
        if partition_name is not None:
            operands.append(partition_id_tensor())
        outs = _bass_exec_p.bind(
            *operands,
            out_avals=tuple(out_avals),
            in_names=tuple(all_names),
            out_names=tuple(out_names),
            lowering_input_output_aliases=(),
            sim_require_finite=True,
            sim_require_nnan=True,
            nc=nc,
        )
        return tuple(outs)

    devices = jax.devices()[:NCORES]
    mesh = Mesh(np.asarray(devices), ("core",))
    sharded = jax.jit(
        shard_map(_body, mesh=mesh,
                  in_specs=(PartitionSpec("core"),) * (n_params + n_outs),
                  out_specs=(PartitionSpec("core"),) * n_outs,
                  check_rep=False),
        donate_argnums=donate,
        keep_unused=True,
    )

    def prep(in_maps):
        concat_in = [
            np.concatenate([np.asarray(m[name]) for m in in_maps], axis=0)
            for name in in_names
        ]
        return concat_in

    def exec_prepped(concat_in):
        concat_zeros = [
            np.zeros((NCORES * s[0], *s[1:]), dt) for s, dt in out_shapes
        ]
        return sharded(*concat_in, *concat_zeros)

    def unpack(out_arrs):
        return [
            {
                name: np.asarray(out_arrs[i]).reshape(
                    NCORES, *out_shapes[i][0])[c]
                for i, name in enumerate(out_names)
            }
            for c in range(NCORES)
        ]

    def run(in_maps):
        return unpack(exec_prepped(prep(in_maps)))

    run.prep = prep
    run.exec_prepped = exec_prepped
    run.unpack = unpack
    run.mesh = mesh
    return run


def get_runner():
    if "run" not in _CACHE:
        _CACHE["run"] = _make_runner(get_nc())
    return _CACHE["run"]


def _f32(v):
    return np.asarray(v, dtype=np.float32)


def _bf(v):
    return np.asarray(v, dtype=np.float32).astype(bfloat16)


def build_rows(xc, yc):
    """Build the four [13, 4096] bf16 row tensors for one batch element.

    Contraction layout (k : L-row      * R-row):
      0-2 : -2*xh_d  * yh_d
      3-5 : -2*xl_d  * yh_d
      6-8 : -2*xh_d  * yl_d
      9   : sqx_h    * 1
      10  : sqx_l    * 1
      11  : 1        * sqy_h
      12  : 1        * sqy_l
    Pass B swaps the roles of x and y with the identical term multiset, so
    P_B = P_A^T up to fp32 accumulation order.
    """
    def side(v):
        vh = _bf(v)
        vl = _bf(_f32(v) - _f32(vh))
        sq = (np.asarray(v, np.float64) ** 2).sum(-1)
        sqh = _bf(sq)
        sql = _bf(sq - np.float64(1.0) * _f32(sqh).astype(np.float64))
        m2h = _bf(-2.0 * _f32(vh))
        m2l = _bf(-2.0 * _f32(vl))
        return vh, vl, sqh, sql, m2h, m2l

    xh, xl, sqxh, sqxl, m2xh, m2xl = side(xc)
    yh, yl, sqyh, sqyl, m2yh, m2yl = side(yc)
    ones = np.ones((N,), dtype=bfloat16)

    def lrows(m2h, m2l, sqh, sql):
        return np.stack([m2h[:, 0], m2h[:, 1], m2h[:, 2],
                         m2l[:, 0], m2l[:, 1], m2l[:, 2],
                         m2h[:, 0], m2h[:, 1], m2h[:, 2],
                         sqh, sql, ones, ones])

    def rrows(vh, vl, sqh, sql):
        return np.stack([vh[:, 0], vh[:, 1], vh[:, 2],
                         vh[:, 0], vh[:, 1], vh[:, 2],
                         vl[:, 0], vl[:, 1], vl[:, 2],
                         ones, ones, sqh, sql])

    return {
        "la": np.ascontiguousarray(lrows(m2xh, m2xl, sqxh, sqxl)),
        "ra": np.ascontiguousarray(rrows(yh, yl, sqyh, sqyl)),
        "lb": np.ascontiguousarray(lrows(m2yh, m2yl, sqyh, sqyl)),
        "rb": np.ascontiguousarray(rrows(xh, xl, sqxh, sqxl)),
    }


def _mins_to_vec(m):
    # m[p, it] is the min for point index it*128 + p
    return np.asarray(m, np.float64).T.reshape(N)


def kernel(x, y, x_mask, y_mask):
    x = np.asarray(x)
    y = np.asarray(y)
    in_maps = [build_rows(x[c], y[c]) for c in range(B)]
    res = get_runner()(in_maps)

    sa = 0.0
    sb = 0.0
    for c in range(B):
        minsA = _mins_to_vec(res[c]["minsA"])  # min over j, per x-point i
        minsB = _mins_to_vec(res[c]["minsB"])  # min over i, per y-point j
        sa += (np.asarray(x_mask[c], np.float64) * minsB).sum()
        sb += (np.asarray(y_mask[c], np.float64) * minsA).sum()
    a = sa / (B * N)
    b = sb / (B * N)
    return np.asarray((a - b) ** 2, dtype=np.float32)


# revision 33
# speedup vs baseline: 331.9743x; 1.0432x over previous
"""Chamfer-loss-overlap kernel for 8 Trainium2 NeuronCores.

Math (per batch element, reference semantics):
    P[i,j] = |x_i|^2 + |y_j|^2 - 2 x_i . y_j          (4096 x 4096)
    a = mean(x_mask * min_i P[i,j])    (min over i, per y-point j)
    b = mean(y_mask * min_j P[i,j])    (min over j, per x-point i)
    out = (a - b)^2

Sharding: batch dim B=8 across the 8 cores (data parallel). Each core
computes its own 4096x4096 distance matrix twice (once per min direction,
so both reductions are along the free dim) and returns the two 4096-long
min vectors. Host applies masks / means in float64 and squares the
difference (scalar combine; the all-reduce of two scalars is trivial).

Device kernel strategy:
  - The distance matrix is produced on TensorE as ONE K=13 bf16 matmul per
    128x512 tile: fp32 x/y are split hi/lo into bf16 (x ~ xh + xl), and the
    |x|^2 / |y|^2 terms ride along as extra contraction rows against ones.
    The per-element error of this split is ~1e-6 relative to P values.
  - Row-mins: ScalarE casts each 2-bank PSUM strip to bf16 in SBUF
    (2x+ cast rate); VectorE reduce-min consumes the 16-bit copy at its
    2x bf16 mode, with 4 strips in flight. Final fold + DMA of the two
    [128, 32] min tensors; masked means run on the host in float64.
"""

import numpy as np
from ml_dtypes import bfloat16

import concourse.bacc as bacc
import concourse.bass as bass
import concourse.mybir as mybir
from concourse import tile

B, N, D = 8, 4096, 3
NCORES = 8
NT = N // 128        # 32 output tiles per pass
QW = 512             # one PSUM bank of fp32
K = 13               # contraction rows of the augmented matmul

# PSUM strip geometry: SW-wide strips, PSUM_BUFS in flight
# (SW * PSUM_BUFS * 4B must be <= 16KB per partition = 8 banks)
SW = 1024
PSUM_BUFS = 4
NSTRIP = N // SW
CPY_BUFS = 6

# Reduction strategy:
#   "cast16b":  ScalarE casts each PSUM strip to bf16 in SBUF (fast 2x+
#               cast), VectorE reduce-min runs at 2x on bf16 (fastest,
#               ~4e-3 extra rel err from bf16 min values)
#   "cast16":   same but fp16 (better precision; ACT cast and reduce both
#               drop to 1x -> slower)
#   "red_psum": VectorE reduce-min directly from PSUM at 1x (safe fallback)
# (tensor_tensor_reduce faults TRN2 hardware on this path - do not use)
REDUCE_MODE = "cast16b"

# Pack the 4 matmuls of each PSUM strip into the 4 32-row groups of the PE
# array (K=13 fits in one group), so they run concurrently (~3x TensorE).
ROW_PACK = False

_CACHE = {}


def _build_nc(reps=1):
    dt = mybir.dt
    nc = bacc.Bacc("TRN2", target_bir_lowering=False, debug=False,
                   num_devices=NCORES)

    la_d = nc.dram_tensor("la", [K, N], dt.bfloat16, kind="ExternalInput")
    ra_d = nc.dram_tensor("ra", [K, N], dt.bfloat16, kind="ExternalInput")
    lb_d = nc.dram_tensor("lb", [K, N], dt.bfloat16, kind="ExternalInput")
    rb_d = nc.dram_tensor("rb", [K, N], dt.bfloat16, kind="ExternalInput")
    minsA_d = nc.dram_tensor("minsA", [128, NT], dt.float32,
                             kind="ExternalOutput")
    minsB_d = nc.dram_tensor("minsB", [128, NT], dt.float32,
                             kind="ExternalOutput")

    with tile.TileContext(nc) as tc:
        with (
            tc.tile_pool(name="rows", bufs=1) as rows,
            tc.tile_pool(name="accs", bufs=1) as accs,
        ):
            npart = 96 + K if ROW_PACK else K
            la = rows.tile([npart, N], dt.bfloat16, tag="la")
            ra = rows.tile([npart, N], dt.bfloat16, tag="ra")
            lb = rows.tile([npart, N], dt.bfloat16, tag="lb")
            rb = rows.tile([npart, N], dt.bfloat16, tag="rb")
            for t, d in ((la, la_d), (ra, ra_d), (lb, lb_d), (rb, rb_d)):
                if ROW_PACK:
                    for r in range(4):
                        nc.sync.dma_start(t[32 * r:32 * r + K, :], d[:])
                else:
                    nc.sync.dma_start(t[:], d[:])

            acc_d = NSTRIP
            accA = accs.tile([128, NT, acc_d], dt.float32, tag="accA")
            accB = accs.tile([128, NT, acc_d], dt.float32, tag="accB")

            import contextlib
            rep_ctx = (tc.For_i(0, reps, 1) if reps > 1
                       else contextlib.nullcontext())
            with rep_ctx:
                _emit_main(nc, tc, la, ra, lb, rb, accA, accB)

            finA = accs.tile([128, NT], dt.float32, tag="finA")
            finB = accs.tile([128, NT], dt.float32, tag="finB")
            nc.vector.tensor_reduce(finA[:], accA[:],
                                    axis=mybir.AxisListType.X,
                                    op=mybir.AluOpType.min)
            nc.vector.tensor_reduce(finB[:], accB[:],
                                    axis=mybir.AxisListType.X,
                                    op=mybir.AluOpType.min)
            nc.sync.dma_start(minsA_d[:], finA[:])
            nc.sync.dma_start(minsB_d[:], finB[:])

    nc.compile()
    return nc


def _emit_main(nc, tc, la, ra, lb, rb, accA, accB):
    dt = mybir.dt
    with (
        tc.tile_pool(name="psum", bufs=PSUM_BUFS,
                     space=bass.MemorySpace.PSUM) as psum,
        tc.tile_pool(name="cpy", bufs=CPY_BUFS) as cpy,
    ):
        cdt = dt.bfloat16 if REDUCE_MODE == "cast16b" else dt.float16
        for L, R, acc in ((la, ra, accA), (lb, rb, accB)):
            for it in range(NT):
                i0 = it * 128
                for h in range(NSTRIP):
                    ps = psum.tile([128, SW], dt.float32, tag="ps", name="ps")
                    for q in range(SW // QW):
                        j0 = h * SW + q * QW
                        nc.tensor.matmul(
                            ps[:, q * QW:(q + 1) * QW],
                            L[:, i0:i0 + 128],
                            R[:, j0:j0 + QW],
                            start=True, stop=True,
                        )
                    if REDUCE_MODE in ("cast16", "cast16b"):
                        cp = cpy.tile([128, SW], cdt, tag="cp", name="cp")
                        nc.scalar.copy(cp[:], ps[:, :])
                        nc.vector.tensor_reduce(
                            acc[:, it, h:h + 1],
                            cp[:],
                            axis=mybir.AxisListType.X,
                            op=mybir.AluOpType.min,
                        )
                    else:
                        nc.vector.tensor_reduce(
                            acc[:, it, h:h + 1],
                            ps[:, :],
                            axis=mybir.AxisListType.X,
                            op=mybir.AluOpType.min,
                        )


def get_nc():
    if "nc" not in _CACHE:
        _CACHE["nc"] = _build_nc()
    return _CACHE["nc"]


def _make_runner(nc):
    """Build a cached jitted SPMD callable for `nc` (one NEFF on all 8
    cores, per-core inputs sharded along axis 0). Mirrors
    bass2jax.run_bass_via_pjrt's multi-core path, but reusable across
    calls so jax tracing/lowering happens once."""
    import jax
    from jax.sharding import Mesh, PartitionSpec
    from jax.experimental.shard_map import shard_map
    from concourse.bass2jax import (
        _bass_exec_p,
        install_neuronx_cc_hook,
        partition_id_tensor,
    )

    install_neuronx_cc_hook()
    partition_name = (nc.partition_id_tensor.name
                      if nc.partition_id_tensor else None)

    in_names = []
    out_names = []
    out_avals = []
    out_shapes = []
    for alloc in nc.m.functions[0].allocations:
        if not isinstance(alloc, mybir.MemoryLocationSet):
            continue
        name = alloc.memorylocations[0].name
        if alloc.kind == "ExternalInput":
            if name != partition_name:
                in_names.append(name)
        elif alloc.kind == "ExternalOutput":
            shape = tuple(alloc.tensor_shape)
            dtype = mybir.dt.np(alloc.dtype)
            out_avals.append(jax.core.ShapedArray(shape, dtype))
            out_names.append(name)
            out_shapes.append((shape, dtype))
    n_params = len(in_names)
    n_outs = len(out_names)
    all_names = list(in_names) + list(out_names)
    if partition_name is not None:
        all_names.append(partition_name)
    donate = tuple(range(n_params, n_params + n_outs))

    def _body(*args):
        operands = list(args)
        if partition_name is not None:
            operands.append(partition_id_tensor())
        outs = _bass_exec_p.bind(
            *operands,
            out_avals=tuple(out_avals),
            in_names=tuple(all_names),
            out_names=tuple(out_names),
            lowering_input_output_aliases=(),
            sim_require_finite=True,
            sim_require_nnan=True,
            nc=nc,
        )
        return tuple(outs)

    devices = jax.devices()[:NCORES]
    mesh = Mesh(np.asarray(devices), ("core",))
    sharded = jax.jit(
        shard_map(_body, mesh=mesh,
                  in_specs=(PartitionSpec("core"),) * (n_params + n_outs),
                  out_specs=(PartitionSpec("core"),) * n_outs,
                  check_rep=False),
        donate_argnums=donate,
        keep_unused=True,
    )

    def prep(in_maps):
        concat_in = [
            np.concatenate([np.asarray(m[name]) for m in in_maps], axis=0)
            for name in in_names
        ]
        return concat_in

    def exec_prepped(concat_in):
        concat_zeros = [
            np.zeros((NCORES * s[0], *s[1:]), dt) for s, dt in out_shapes
        ]
        return sharded(*concat_in, *concat_zeros)

    def unpack(out_arrs):
        return [
            {
                name: np.asarray(out_arrs[i]).reshape(
                    NCORES, *out_shapes[i][0])[c]
                for i, name in enumerate(out_names)
            }
            for c in range(NCORES)
        ]

    def run(in_maps):
        return unpack(exec_prepped(prep(in_maps)))

    run.prep = prep
    run.exec_prepped = exec_prepped
    run.unpack = unpack
    run.mesh = mesh
    return run


def get_runner():
    if "run" not in _CACHE:
        _CACHE["run"] = _make_runner(get_nc())
    return _CACHE["run"]


def _f32(v):
    return np.asarray(v, dtype=np.float32)


def _bf(v):
    return np.asarray(v, dtype=np.float32).astype(bfloat16)


def build_rows(xc, yc):
    """Build the four [13, 4096] bf16 row tensors for one batch element.

    Contraction layout (k : L-row      * R-row):
      0-2 : -2*xh_d  * yh_d
      3-5 : -2*xl_d  * yh_d
      6-8 : -2*xh_d  * yl_d
      9   : sqx_h    * 1
      10  : sqx_l    * 1
      11  : 1        * sqy_h
      12  : 1        * sqy_l
    Pass B swaps the roles of x and y with the identical term multiset, so
    P_B = P_A^T up to fp32 accumulation order.
    """
    def side(v):
        vh = _bf(v)
        vl = _bf(_f32(v) - _f32(vh))
        sq = (np.asarray(v, np.float64) ** 2).sum(-1)
        sqh = _bf(sq)
        sql = _bf(sq - np.float64(1.0) * _f32(sqh).astype(np.float64))
        m2h = _bf(-2.0 * _f32(vh))
        m2l = _bf(-2.0 * _f32(vl))
        return vh, vl, sqh, sql, m2h, m2l

    xh, xl, sqxh, sqxl, m2xh, m2xl = side(xc)
    yh, yl, sqyh, sqyl, m2yh, m2yl = side(yc)
    ones = np.ones((N,), dtype=bfloat16)

    def lrows(m2h, m2l, sqh, sql):
        return np.stack([m2h[:, 0], m2h[:, 1], m2h[:, 2],
                         m2l[:, 0], m2l[:, 1], m2l[:, 2],
                         m2h[:, 0], m2h[:, 1], m2h[:, 2],
                         sqh, sql, ones, ones])

    def rrows(vh, vl, sqh, sql):
        return np.stack([vh[:, 0], vh[:, 1], vh[:, 2],
                         vh[:, 0], vh[:, 1], vh[:, 2],
                         vl[:, 0], vl[:, 1], vl[:, 2],
                         ones, ones, sqh, sql])

    return {
        "la": np.ascontiguousarray(lrows(m2xh, m2xl, sqxh, sqxl)),
        "ra": np.ascontiguousarray(rrows(yh, yl, sqyh, sqyl)),
        "lb": np.ascontiguousarray(lrows(m2yh, m2yl, sqyh, sqyl)),
        "rb": np.ascontiguousarray(rrows(xh, xl, sqxh, sqxl)),
    }


def _mins_to_vec(m):
    # m[p, it] is the min for point index it*128 + p
    return np.asarray(m, np.float64).T.reshape(N)


def kernel(x, y, x_mask, y_mask):
    x = np.asarray(x)
    y = np.asarray(y)
    in_maps = [build_rows(x[c], y[c]) for c in range(B)]
    res = get_runner()(in_maps)

    sa = 0.0
    sb = 0.0
    for c in range(B):
        minsA = _mins_to_vec(res[c]["minsA"])  # min over j, per x-point i
        minsB = _mins_to_vec(res[c]["minsB"])  # min over i, per y-point j
        sa += (np.asarray(x_mask[c], np.float64) * minsB).sum()
        sb += (np.asarray(y_mask[c], np.float64) * minsA).sum()
    a = sa / (B * N)
    b = sb / (B * N)
    return np.asarray((a - b) ** 2, dtype=np.float32)
